# revision 1
# baseline (speedup 1.0000x reference)
"""CODAPromptPool kernel for 8 Trainium2 NeuronCores.

Reference computation (per batch element b):
    query  = mean(x[b], axis=0)                      # [D]
    sim    = l2norm(query) @ l2norm(e_keys).T        # [POOL]
    top4   = top_k(sim, 4) indices (descending)
    out[b] = concat([g_prompts[task_id],             # rows 0..7
                     e_prompts[top4].reshape(32, D), # rows 8..39
                     cls_token,                      # row 40
                     x[b]], axis=0)                  # rows 41..2088

Sharding: data-parallel over batch (64 /8 cores = 8 per core); the pool /
keys / g / cls are replicated. The kernel is HBM-bound by the x copy
(48 MiB in + 49 MiB out per core), so x is streamed through SBUF once:
each tile is DMA'd in, added into a per-batch accumulator (for the mean)
and DMA'd straight out to its slot in the output. Routing notes:
  * top-k ranking is invariant to positive per-row scaling, so neither
    the division by S (mean) nor the query l2-normalization is needed —
    only the keys must be normalized.
  * the gather of selected prompt blocks uses indirect DMA with the
    uint32 indices produced by the DVE max8/max_index instructions.
"""

import numpy as np

import concourse.bacc as bacc
import concourse.bass as bass
import concourse.mybir as mybir
from concourse import bass_utils
from concourse._compat import get_trn_type
from concourse.masks import make_identity
from concourse.tile import TileContext

F32 = mybir.dt.float32
U32 = mybir.dt.uint32

NCORES = 8
B, S, D = 64, 2048, 768
BC = B // NCORES                 # batches per core
POOL, L, TOPK = 32, 8, 4
E_OFF = L                        # selected blocks start row
CLS_ROW = L + TOPK * L           # 40
X_OFF = CLS_ROW + 1              # 41
OUTS = X_OFF + S                 # 2089
EPS = 1e-12
P = 128

PROFILE = False                  # test harness sets True for NTFF tracing
LAST_RESULT = None               # BassKernelResults of the last run


def build(bc=BC, s=S, debug=False, per_batch=False, defer=2, xp_bufs=16):
    assert s % P == 0 and s // P >= 2
    nt = s // P                  # seq tiles per batch
    ndc = D // P                 # 6 D-chunks of 128
    outs = X_OFF + s
    x = mybir.AxisListType.X

    nc = bacc.Bacc(get_trn_type() or "TRN2", target_bir_lowering=False, debug=debug)
    x_h = nc.declare_dram_parameter("x", [bc, s, D], F32, isOutput=False)
    ep_h = nc.declare_dram_parameter("e_prompts", [POOL, L * D], F32, isOutput=False)
    ek_h = nc.declare_dram_parameter("e_keys", [POOL, D], F32, isOutput=False)
    g_h = nc.declare_dram_parameter("g_rep", [bc, L, D], F32, isOutput=False)
    cls_h = nc.declare_dram_parameter("cls_rep", [bc, 1, D], F32, isOutput=False)
    out_h = nc.declare_dram_parameter("out", [bc, outs, D], F32, isOutput=True)

    with TileContext(nc) as tc:
        with (
            tc.tile_pool(name="consts", bufs=1) as consts,
            tc.tile_pool(name="xp", bufs=xp_bufs) as xp,
            tc.tile_pool(name="xdef", bufs=1) as xdef,
            tc.tile_pool(name="accp", bufs=2) as accp,
            tc.tile_pool(name="rt", bufs=2) as rt,
            tc.tile_pool(name="gp", bufs=1) as gp,
            tc.tile_pool(name="ps", bufs=2, space="PSUM") as ps,
            tc.tile_pool(name="ps1", bufs=1, space="PSUM") as ps1,
        ):
            # Routing-independent header rows, straight DRAM->DRAM.
            nc.gpsimd.dma_start(out_h[:, 0:L, :], g_h[:])
            nc.gpsimd.dma_start(out_h[:, CLS_ROW : CLS_ROW + 1, :], cls_h[:])

            ident = consts.tile([P, P], F32)
            make_identity(nc, ident[:])

            # Normalized keys, transposed to [D-chunk partitions, POOL].
            keys = consts.tile([POOL, D], F32)
            nc.sync.dma_start(keys[:], ek_h[:])
            sq = consts.tile([POOL, D], F32)
            nc.vector.tensor_mul(sq[:], keys[:], keys[:])
            n2 = consts.tile([POOL, 1], F32)
            nc.vector.reduce_sum(n2[:], sq[:], axis=x)
            eps = consts.tile([POOL, 1], F32)
            nc.vector.memset(eps[:], EPS)
            nrm = consts.tile([POOL, 1], F32)
            nc.scalar.activation(
                nrm[:], n2[:], mybir.ActivationFunctionType.Sqrt, bias=eps[:, 0:1]
            )
            rk = consts.tile([POOL, 1], F32)
            nc.vector.reciprocal(rk[:], nrm[:])
            kn = consts.tile([P, D], F32)
            nc.vector.memset(kn[:], 0.0)
            nc.vector.tensor_scalar_mul(kn[0:POOL, :], keys[:], rk[:, 0:1])
            knT = consts.tile([P, ndc * POOL], F32)
            for c in range(ndc):
                pt = ps.tile([P, P], F32, tag="tp")
                nc.tensor.transpose(pt[:], kn[:, bass.ts(c, P)], ident[:])
                nc.vector.tensor_copy(knT[:, bass.ts(c, POOL)], pt[:, 0:POOL])

            # Stream x through SBUF: accumulate seq-sum + copy to output.
            # Routing + gather run per batch as soon as that batch's sum is
            # complete, so only the last batch's short chain sits at the end
            # of the stream. The last batch's tiles stay resident in SBUF and
            # their output writes are emitted LAST, so the write stream keeps
            # the DMA fabric saturated while that final chain runs.
            n_def = int(defer)
            def_start = bc - n_def
            def_tiles = {}
            qt_all = None if per_batch else consts.tile([P, ndc * bc], F32)
            for b in range(bc):
                acc = accp.tile([P, D], F32, tag="acc")
                first = None
                for t in range(nt):
                    if b >= def_start:
                        xt = xdef.tile([P, D], F32, tag=f"bdef_{b}_{t}")
                        def_tiles[(b, t)] = xt
                    else:
                        xt = xp.tile([P, D], F32, tag="xt")
                    # During the first batch the write stream has no work yet,
                    # so pull input on both HWDGE rings to shorten the ramp.
                    in_eng = nc.scalar if (b == 0 and t % 2 == 1) else nc.sync
                    in_eng.dma_start(xt[:], x_h[b, bass.ts(t, P), :])
                    if b < def_start:
                        nc.scalar.dma_start(
                            out_h[b, X_OFF + t * P : X_OFF + (t + 1) * P, :], xt[:]
                        )
                    if t == 0:
                        first = xt
                    elif t == 1:
                        nc.vector.tensor_add(acc[:], first[:], xt[:])
                    else:
                        nc.vector.tensor_add(acc[:], acc[:], xt[:])
                # Partition-reduce acc via PE transpose + free-axis sum.
                if per_batch:
                    qt = rt.tile([P, ndc], F32, tag="qt")
                else:
                    qt = qt_all
                for c in range(ndc):
                    pt = ps.tile([P, P], F32, tag="tp")
                    nc.tensor.transpose(pt[:], acc[:, bass.ts(c, P)], ident[:])
                    col = c if per_batch else c * bc + b
                    nc.vector.reduce_sum(qt[:, col : col + 1], pt[:], axis=x)
                if not per_batch:
                    continue
                # similarity [1, POOL] for this batch, contracted over D.
                sps = ps1.tile([1, POOL], F32, tag="s")
                for c in range(ndc):
                    nc.tensor.matmul(
                        sps[:],
                        lhsT=qt[:, c : c + 1],
                        rhs=knT[:, bass.ts(c, POOL)],
                        start=(c == 0),
                        stop=(c == ndc - 1),
                    )
                s_sb = rt.tile([1, POOL], F32, tag="ssb")
                nc.vector.tensor_copy(s_sb[:], sps[:])
                mx = rt.tile([1, 8], F32, tag="mx")
                ix = rt.tile([1, 8], U32, tag="ix")
                nc.vector.max_with_indices(mx[:], ix[:], s_sb[:])
                # Spread top-4 indices to one partition each, gather the four
                # [L, D] blocks, write them to this batch's header region.
                ixt = rt.tile([TOPK, 1], U32, tag="ixt")
                nc.gpsimd.dma_start(ixt[:], ix[0:1, 0:TOPK])
                gth = gp.tile([TOPK, L * D], F32, tag="gth")
                nc.gpsimd.indirect_dma_start(
                    out=gth[:],
                    out_offset=None,
                    in_=ep_h[:],
                    in_offset=bass.IndirectOffsetOnAxis(ap=ixt[:, 0:1], axis=0),
                )
                e_dst = out_h[b, E_OFF : E_OFF + TOPK * L, :].rearrange(
                    "(k l) d -> k (l d)", k=TOPK
                )
                nc.sync.dma_start(e_dst, gth[:])

            if not per_batch:
                # Batched routing for all bc batches at once.
                sps = ps1.tile([bc, POOL], F32, tag="s")
                for c in range(ndc):
                    nc.tensor.matmul(
                        sps[:],
                        lhsT=qt_all[:, bass.ts(c, bc)],
                        rhs=knT[:, bass.ts(c, POOL)],
                        start=(c == 0),
                        stop=(c == ndc - 1),
                    )
                s_sb = rt.tile([bc, POOL], F32, tag="ssb")
                nc.vector.tensor_copy(s_sb[:], sps[:])
                mx = rt.tile([bc, 8], F32, tag="mx")
                ix = rt.tile([bc, 8], U32, tag="ix")
                nc.vector.max_with_indices(mx[:], ix[:], s_sb[:])
                idx32 = rt.tile([bc * TOPK, 1], U32, tag="idx32")
                nc.gpsimd.dma_start(idx32[:], ix[:, 0:TOPK])
                gth = gp.tile([bc * TOPK, L * D], F32, tag="gth")
                nc.gpsimd.indirect_dma_start(
                    out=gth[:],
                    out_offset=None,
                    in_=ep_h[:],
                    in_offset=bass.IndirectOffsetOnAxis(ap=idx32[:, 0:1], axis=0),
                )

            # Deferred output writes for the last n_def batches, split across
            # both HWDGE rings so they drain at full rate while the routing
            # chain (max8 -> index spread -> indirect gather) runs. The gather
            # write goes last on sync so it can't head-of-line-block them.
            for i, ((b, t), xt) in enumerate(sorted(def_tiles.items())):
                eng = nc.scalar if i % 2 == 0 else nc.sync
                eng.dma_start(
                    out_h[b, X_OFF + t * P : X_OFF + (t + 1) * P, :], xt[:]
                )
            if not per_batch:
                e_dst = out_h[:, E_OFF : E_OFF + TOPK * L, :].rearrange(
                    "b (k l) d -> b k (l d)", k=TOPK
                )
                half = (bc // 2) * TOPK
                nc.sync.dma_start(e_dst[0 : bc // 2], gth[0:half, :])
                nc.scalar.dma_start(e_dst[bc // 2 : bc], gth[half:, :])

    nc.compile()
    return nc


_NC_CACHE: dict = {}


def _get_nc(bc=BC, s=S):
    key = (bc, s)
    if key not in _NC_CACHE:
        _NC_CACHE[key] = build(bc, s)
    return _NC_CACHE[key]


def kernel(x, g_prompts, e_prompts, e_keys, cls_token, task_id):
    global LAST_RESULT
    nc = _get_nc()
    tid = int(np.asarray(task_id))
    x = np.ascontiguousarray(np.asarray(x, dtype=np.float32))
    g_rep = np.ascontiguousarray(
        np.broadcast_to(np.asarray(g_prompts, np.float32)[tid][None], (BC, L, D))
    )
    cls_rep = np.ascontiguousarray(
        np.broadcast_to(np.asarray(cls_token, np.float32).reshape(1, 1, D), (BC, 1, D))
    )
    ep = np.ascontiguousarray(np.asarray(e_prompts, np.float32).reshape(POOL, L * D))
    ek = np.ascontiguousarray(np.asarray(e_keys, np.float32))

    in_maps = [
        {
            "x": x[c * BC : (c + 1) * BC],
            "e_prompts": ep,
            "e_keys": ek,
            "g_rep": g_rep,
            "cls_rep": cls_rep,
        }
        for c in range(NCORES)
    ]
    res = bass_utils.run_bass_kernel_spmd(
        nc, in_maps, list(range(NCORES)), trace=PROFILE
    )
    LAST_RESULT = res
    return np.concatenate([res.results[c]["out"] for c in range(NCORES)], axis=0)

